# revision 14
# baseline (speedup 1.0000x reference)
"""Trainium2 Bass kernel for DocumentBertScoringLoss (B=8192).

loss = MSE(p, g) + MR(p, g) + SIM(p, g), returned as shape-(1,) fp32.

Margin-ranking identity (ties included):

    sum_{m,n} max(0, 0.1 - r*dp) = 0.1*B^2 - 2*S,
    S = sum_{i<j} min(p_(j) - p_(i), 0.1)   on the sorted predictions.

S is evaluated with a banded sweep over the sorted vector: pair (i, j),
i<j, is assigned to the 128-row chunk containing i and computed
explicitly only when j < chunk_start + W.  The host verifies the band
bound  ps[s+W] - ps[s+127] >= 0.1  for every chunk start s (every
skipped pair then differs by >= 0.1 and contributes exactly 0.1, a
closed-form count added on the host); if the bound fails, W widens
(ladder up to W=B, the exact dense sweep, so the kernel is correct for
arbitrary inputs).  In-window pairs with j <= i clamp to 0 and rows
past B are padded with -1000 (clamp 0), so each 128xW rectangle sums
exactly its i<j near pairs plus W*s1 per row, removed via one
correction.

Device pipeline per core (1024 sorted rows):
  - DVE: 8x tensor_scalar h = min(max(X, s1), s1+0.1) over [128, W]
    fp16 windows (4x perf mode, ~0.26 ns/col).
  - PE reduces each h column-wise using h as matmul *weights* against a
    ones vector ([128,128] pieces accumulating into PSUM [128,1]), then
    folds partitions with two more 1-col matmuls.
  - MSE/SIM: dot/pp/gg via 3 scalar_tensor_tensor accumulates off the
    packed p||g tile; 1/sqrt(pp*gg) via one ACT Rsqrt (single act-table
    load); short DVE scalar chain reading PSUM directly.
  - DMAs: X broadcast in 3 pieces + prow/pg/out spread over the two
    HWDGE queues and Pool SWDGE to overlap the ~2.2us per-DMA latency
    (desc-gen + dge delay + 900ns completion semaphore).
Host gather sums the 8 per-core scalars and adds the far-pair constant.
"""

import numpy as np

import concourse.bass as bass
import concourse.bacc as bacc
import concourse.mybir as mybir
from concourse.bass_utils import run_bass_kernel_spmd
from concourse.tile import TileContext
from concourse.alu_op_type import AluOpType

B = 8192
NCORES = 8
ROWS_PER_CORE = B // NCORES          # 1024
NCHUNK = ROWS_PER_CORE // 128        # 8 chunks of 128 partitions
MR_BIAS = 0.1
PAD_VAL = -1000.0

# Band-width ladder: W=512 holds for the reference N(0,1) draw (host
# verified per call); later entries are correctness fallbacks (W=B is
# the exact dense sweep).
W_LADDER = (512, 640, 1024, 1536, 2560, 4096, 8192)

F32 = mybir.dt.float32
F16 = mybir.dt.float16

_CACHED = {}


def _build_nc(W):
    WIN = 128 * (NCHUNK - 1) + W     # per-core broadcast window size
    nc = bacc.Bacc("TRN2", target_bir_lowering=False, debug=False,
                   num_devices=NCORES)

    xwin_d = nc.dram_tensor("x_win", [WIN], F16, kind="ExternalInput")
    prow_d = nc.dram_tensor("p_rows", [128, 2 * NCHUNK], F32, kind="ExternalInput")
    pg_d = nc.dram_tensor("pg", [128, 256], F16, kind="ExternalInput")
    out_d = nc.dram_tensor("out", [2], F32, kind="ExternalOutput")

    AF = mybir.ActivationFunctionType
    K2 = -2.0 / (float(B) * float(B))

    # X piece split: A covers chunk 0 (+1 at W=512), B the middle, C the tail.
    CUT1 = W + 128
    CUT2 = W + 128 * 5

    with TileContext(nc) as tc:
        with (
            tc.tile_pool(name="const", bufs=1) as cpool,
            tc.tile_pool(name="hbuf", bufs=3) as hpool,
            tc.tile_pool(name="psum", bufs=1, space="PSUM") as ppool,
        ):
            xbf = cpool.tile([128, WIN], F16, name="xbf")
            prow = cpool.tile([128, 2 * NCHUNK], F32, name="prow")
            pg16 = cpool.tile([128, 256], F16, name="pg16")
            ones16 = cpool.tile([128, 1], F16, name="ones16")
            ones32 = cpool.tile([128, 1], F32, name="ones32")
            zeros1 = cpool.tile([128, 1], F32, name="zeros1")
            stacked = cpool.tile([128, 3], F32, name="stacked")
            junk_stt = cpool.tile([128, 64], F16, name="junk_stt")
            s1sum = cpool.tile([128, 1], F32, name="s1sum")
            v_sb = cpool.tile([128, 1], F32, name="v_sb")
            sc = cpool.tile([1, 16], F32, name="sc")
            out_sb = cpool.tile([1, 2], F32, name="out_sb")

            psum_acc = ppool.tile([128, 1], F32, name="psum_acc")
            psum_small = ppool.tile([1, 3], F32, name="psum_small")
            psum_tot = ppool.tile([1, 1], F32, name="psum_tot")

            # ---- input DMAs ----
            # HWDGE (one serial ~630ns/desc-gen resource fed by the SP and
            # ACT queues): X pieces in consumption order.  Pool SWDGE
            # (idle engine) carries prow, pg, and preps the out DMA early.
            xw = xwin_d[:]
            nc.sync.dma_start(xbf[:, 0:CUT1], xw[0:CUT1].partition_broadcast(128))
            nc.scalar.dma_start(
                xbf[:, CUT1:CUT2], xw[CUT1:CUT2].partition_broadcast(128)
            )
            nc.gpsimd.dma_start(prow, prow_d[:, :])
            nc.gpsimd.dma_start(
                xbf[:, CUT2:WIN], xw[CUT2:WIN].partition_broadcast(128)
            )
            nc.sync.dma_start(pg16, pg_d[:, :])

            nc.vector.memset(ones16, 1.0)
            nc.vector.memset(ones32, 1.0)
            nc.vector.memset(zeros1, 0.0)
            k2vec = cpool.tile([128, 1], F32, name="k2vec")
            nc.vector.memset(k2vec, K2)
            # Dummy sqrt so the single act-table load (during the DMA
            # wait) covers the one real ACT op.
            warm_rs = cpool.tile([1, 1], F32, name="warm_rs")
            nc.scalar.activation(warm_rs, ones32[0:1, :], AF.Sqrt,
                                 bias=zeros1[0:1, :])

            # ---- banded clamp sweep (kept ahead of everything else in
            # the DVE stream via scheduler priority) ----
            mm = 0
            total_mm = NCHUNK * (W // 128)
            with tc.high_priority():
                for c in range(NCHUNK):
                    h = hpool.tile([128, W], F16, tag="h", name="h", bufs=8)
                    nc.vector.tensor_scalar(
                        h, xbf[:, 128 * c:128 * c + W],
                        prow[:, c:c + 1], prow[:, NCHUNK + c:NCHUNK + c + 1],
                        AluOpType.max, AluOpType.min,
                    )
                    for j in range(0, W, 128):
                        nc.tensor.matmul(
                            psum_acc, h[:, j:j + 128], ones16,
                            start=(mm == 0), stop=(mm == total_mm - 1),
                            skip_group_check=True,
                        )
                        mm += 1

            # ---- MR tail: v = psum_acc - W*s1sum, out1 = K2 * sum(v)
            # (K2 folded into the reduction vector; the final PSUM read is
            # a plain copy since stt/ts cannot read PSUM here) ----
            nc.vector.tensor_reduce(
                s1sum, prow[:, 0:NCHUNK], mybir.AxisListType.X, AluOpType.add
            )
            acc_sb = cpool.tile([128, 1], F32, name="acc_sb")
            nc.vector.tensor_copy(acc_sb, psum_acc)
            nc.vector.scalar_tensor_tensor(
                v_sb, s1sum, -float(W), acc_sb,
                AluOpType.mult, AluOpType.add,
            )
            nc.tensor.matmul(psum_tot, v_sb, k2vec, start=True, stop=True)
            nc.vector.tensor_copy(out_sb[0:1, 1:2], psum_tot[0:1, 0:1])

            # ---- small terms: dot/pp/gg off the packed p||g tile ----
            p_ap = pg16[:, 0:64]
            g_ap = pg16[:, 64:128]
            nc.vector.scalar_tensor_tensor(
                junk_stt, p_ap, 1.0, g_ap, AluOpType.mult, AluOpType.mult,
                accum_out=stacked[:, 0:1],
            )
            junk_sq = cpool.tile([128, 64], F32, name="junk_sq")
            nc.scalar.activation(
                junk_sq, p_ap, AF.Square, bias=zeros1,
                accum_out=stacked[:, 1:2],
            )
            nc.scalar.activation(
                junk_sq, g_ap, AF.Square, bias=zeros1,
                accum_out=stacked[:, 2:3],
            )
            nc.tensor.matmul(psum_small, ones32, stacked, start=True, stop=True)

            # ---- scalar chain (partition 0; ts reads PSUM directly;
            # the (1 - .)/8 constant is added on the host) ----
            dot = psum_small[0:1, 0:1]
            pp = psum_small[0:1, 1:2]
            gg = psum_small[0:1, 2:3]
            prodc = sc[0:1, 1:2]
            nc.vector.tensor_scalar(
                prodc, pp, gg, 1e-16, AluOpType.mult, AluOpType.max
            )
            denom = sc[0:1, 7:8]
            nc.scalar.activation(denom, prodc, AF.Sqrt, bias=zeros1[0:1, :])
            inv = sc[0:1, 2:3]
            nc.vector.reciprocal(inv, denom)
            t1 = sc[0:1, 3:4]
            nc.vector.tensor_scalar(
                t1, pp, gg, dot, AluOpType.add, AluOpType.subtract
            )
            t2 = sc[0:1, 4:5]
            nc.vector.tensor_scalar(
                t2, t1, dot, 1.0 / (8.0 * B), AluOpType.subtract, AluOpType.mult
            )
            sims = sc[0:1, 6:7]
            nc.vector.tensor_scalar(
                sims, inv, dot, -1.0 / 8.0, AluOpType.mult, AluOpType.mult
            )
            nc.vector.scalar_tensor_tensor(
                out_sb[0:1, 0:1], sims, 1.0, t2,
                AluOpType.mult, AluOpType.add,
            )
            nc.sync.dma_start(out_d[None, :], out_sb)

    nc.compile()
    return nc


def _pick_w(ps):
    starts = np.arange(0, B, 128)
    for W in W_LADDER:
        s = starts[starts + W < B]
        if s.size == 0 or np.all(ps[s + W] - ps[s + 127] >= MR_BIAS):
            return W
    return B


def kernel(predictions: np.ndarray, correct_output: np.ndarray) -> np.ndarray:
    p = np.ascontiguousarray(np.asarray(predictions, dtype=np.float32))
    g = np.ascontiguousarray(np.asarray(correct_output, dtype=np.float32))

    ps = np.sort(p)
    W = _pick_w(ps)
    if W not in _CACHED:
        _CACHED[W] = _build_nc(W)
    nc = _CACHED[W]

    WIN = 128 * (NCHUNK - 1) + W
    ps16 = np.full(B + WIN, PAD_VAL, dtype=np.float16)
    ps16[:B] = ps.astype(np.float16)
    pg = np.zeros((128, 256), dtype=np.float16)
    pg[:, 0:64] = p.reshape(128, 64)
    pg[:, 64:128] = g.reshape(128, 64)

    in_maps = []
    for k in range(NCORES):
        r0 = k * ROWS_PER_CORE
        in_maps.append(
            {
                "x_win": ps16[r0:r0 + WIN].copy(),
                "p_rows": np.ascontiguousarray(np.concatenate(
                    [ps[r0:r0 + ROWS_PER_CORE].reshape(NCHUNK, 128).T,
                     ps[r0:r0 + ROWS_PER_CORE].reshape(NCHUNK, 128).T
                     + np.float32(MR_BIAS)], axis=1
                )),
                "pg": pg,
            }
        )

    res = None
    last_exc = None
    for _attempt in range(3):
        try:
            res = run_bass_kernel_spmd(nc, in_maps, core_ids=list(range(NCORES)))
            break
        except Exception as e:  # transient NRT/axon device errors
            last_exc = e
            import time as _time
            _time.sleep(1.0)
    if res is None:
        raise last_exc

    # Host gather: per-core scalars + closed-form far-pair constant.
    #   mr = 0.1 - (2/B^2) * (S_near + 0.1*N_far)
    # device out[1] carries -(2/B^2)*S_near_share, out[0] (mse+sim)/8.
    i = np.arange(B, dtype=np.int64)
    hi = np.minimum(128 * (i // 128) + W, B)
    n_near = int(np.sum(hi - i - 1))
    n_far = B * (B - 1) // 2 - n_near
    mr_const = MR_BIAS - 2.0 * MR_BIAS * n_far / (float(B) * float(B))

    total = np.float64(mr_const) + 1.0
    for r in res.results:
        total += np.float64(r["out"][0]) + np.float64(r["out"][1])
    return np.array([total], dtype=np.float32)


if __name__ == "__main__":
    rng = np.random.default_rng(0)
    p = rng.standard_normal(B).astype(np.float32)
    g = rng.standard_normal(B).astype(np.float32)
    print(kernel(p, g))


# revision 15
# speedup vs baseline: 1.0383x; 1.0383x over previous
"""Trainium2 Bass kernel for DocumentBertScoringLoss (B=8192).

loss = MSE(p, g) + MR(p, g) + SIM(p, g), returned as shape-(1,) fp32.

Margin-ranking identity (ties included):

    sum_{m,n} max(0, 0.1 - r*dp) = 0.1*B^2 - 2*S,
    S = sum_{i<j} min(p_(j) - p_(i), 0.1)   on the sorted predictions.

S is evaluated with a banded sweep over the sorted vector: pair (i, j),
i<j, is assigned to the 128-row chunk containing i and computed
explicitly only when j < chunk_start + W.  The host verifies the band
bound  ps[s+W] - ps[s+127] >= 0.1  for every chunk start s (every
skipped pair then differs by >= 0.1 and contributes exactly 0.1, a
closed-form count added on the host); if the bound fails, W widens
(ladder up to W=B, the exact dense sweep, so the kernel is correct for
arbitrary inputs).  In-window pairs with j <= i clamp to 0 and rows
past B are padded with -1000 (clamp 0), so each 128xW rectangle sums
exactly its i<j near pairs plus W*s1 per row, removed via one
correction.

Device pipeline per core (1024 sorted rows):
  - DVE: 8x tensor_scalar h = min(max(X, s1), s1+0.1) over [128, W]
    fp16 windows (4x perf mode, ~0.26 ns/col).
  - PE reduces each h column-wise using h as matmul *weights* against a
    ones vector ([128,128] pieces accumulating into PSUM [128,1]), then
    folds partitions with two more 1-col matmuls.
  - MSE/SIM: dot/pp/gg via 3 scalar_tensor_tensor accumulates off the
    packed p||g tile; 1/sqrt(pp*gg) via one ACT Rsqrt (single act-table
    load); short DVE scalar chain reading PSUM directly.
  - DMAs: X broadcast in 3 pieces + prow/pg/out spread over the two
    HWDGE queues and Pool SWDGE to overlap the ~2.2us per-DMA latency
    (desc-gen + dge delay + 900ns completion semaphore).
Host gather sums the 8 per-core scalars and adds the far-pair constant.
"""

import numpy as np

import concourse.bass as bass
import concourse.bacc as bacc
import concourse.mybir as mybir
from concourse.bass_utils import run_bass_kernel_spmd
from concourse.tile import TileContext
from concourse.alu_op_type import AluOpType

B = 8192
NCORES = 8
ROWS_PER_CORE = B // NCORES          # 1024
NCHUNK = ROWS_PER_CORE // 128        # 8 chunks of 128 partitions
MR_BIAS = 0.1
PAD_VAL = -1000.0

# Band-width ladder: W=512 holds for the reference N(0,1) draw (host
# verified per call); later entries are correctness fallbacks (W=B is
# the exact dense sweep).
W_LADDER = (512, 640, 1024, 1536, 2560, 4096, 8192)

F32 = mybir.dt.float32
F16 = mybir.dt.float16

_CACHED = {}


def _build_nc(W):
    WIN = 128 * (NCHUNK - 1) + W     # per-core broadcast window size
    nc = bacc.Bacc("TRN2", target_bir_lowering=False, debug=False,
                   num_devices=NCORES)

    xwin_d = nc.dram_tensor("x_win", [WIN], F16, kind="ExternalInput")
    prow_d = nc.dram_tensor("p_rows", [128, 2 * NCHUNK], F32, kind="ExternalInput")
    pg_d = nc.dram_tensor("pg", [128, 256], F16, kind="ExternalInput")
    out_d = nc.dram_tensor("out", [2], F32, kind="ExternalOutput")

    AF = mybir.ActivationFunctionType
    K2 = -2.0 / (float(B) * float(B))

    # X piece split: A covers chunks 0-2, B the rest (no third piece).
    CUT1 = W + 128 * 2
    CUT2 = WIN

    with TileContext(nc) as tc:
        with (
            tc.tile_pool(name="const", bufs=1) as cpool,
            tc.tile_pool(name="hbuf", bufs=3) as hpool,
            tc.tile_pool(name="psum", bufs=1, space="PSUM") as ppool,
        ):
            xbf = cpool.tile([128, WIN], F16, name="xbf")
            prow = cpool.tile([128, 2 * NCHUNK], F32, name="prow")
            pg16 = cpool.tile([128, 256], F16, name="pg16")
            ones16 = cpool.tile([128, 1], F16, name="ones16")
            ones32 = cpool.tile([128, 1], F32, name="ones32")
            zeros1 = cpool.tile([128, 1], F32, name="zeros1")
            stacked = cpool.tile([128, 3], F32, name="stacked")
            junk_stt = cpool.tile([128, 64], F16, name="junk_stt")
            s1sum = cpool.tile([128, 1], F32, name="s1sum")
            v_sb = cpool.tile([128, 1], F32, name="v_sb")
            sc = cpool.tile([1, 16], F32, name="sc")
            out_sb = cpool.tile([1, 2], F32, name="out_sb")

            psum_acc = ppool.tile([128, 1], F32, name="psum_acc")
            psum_small = ppool.tile([1, 3], F32, name="psum_small")
            psum_tot = ppool.tile([1, 1], F32, name="psum_tot")

            # ---- input DMAs ----
            # HWDGE (one serial ~630ns/desc-gen resource fed by the SP and
            # ACT queues): X pieces in consumption order.  Pool SWDGE
            # (idle engine) carries prow, pg, and preps the out DMA early.
            xw = xwin_d[:]
            nc.sync.dma_start(xbf[:, 0:CUT1], xw[0:CUT1].partition_broadcast(128))
            nc.scalar.dma_start(
                xbf[:, CUT1:CUT2], xw[CUT1:CUT2].partition_broadcast(128)
            )
            nc.gpsimd.dma_start(prow, prow_d[:, :])
            nc.sync.dma_start(pg16, pg_d[:, :])

            nc.vector.memset(ones16, 1.0)
            nc.vector.memset(ones32, 1.0)
            nc.vector.memset(zeros1, 0.0)
            k2vec = cpool.tile([128, 1], F32, name="k2vec")
            nc.vector.memset(k2vec, K2)
            # Dummy sqrt so the single act-table load (during the DMA
            # wait) covers the one real ACT op.
            warm_rs = cpool.tile([1, 1], F32, name="warm_rs")
            nc.scalar.activation(warm_rs, ones32[0:1, :], AF.Sqrt,
                                 bias=zeros1[0:1, :])

            # ---- banded clamp sweep (kept ahead of everything else in
            # the DVE stream via scheduler priority) ----
            mm = 0
            total_mm = NCHUNK * (W // 128)
            with tc.high_priority():
                for c in range(NCHUNK):
                    h = hpool.tile([128, W], F16, tag="h", name="h", bufs=8)
                    nc.vector.tensor_scalar(
                        h, xbf[:, 128 * c:128 * c + W],
                        prow[:, c:c + 1], prow[:, NCHUNK + c:NCHUNK + c + 1],
                        AluOpType.max, AluOpType.min,
                    )
                    for j in range(0, W, 128):
                        nc.tensor.matmul(
                            psum_acc, h[:, j:j + 128], ones16,
                            start=(mm == 0), stop=(mm == total_mm - 1),
                            skip_group_check=True,
                        )
                        mm += 1

            # ---- MR tail: v = psum_acc - W*s1sum, out1 = K2 * sum(v)
            # (K2 folded into the reduction vector; the final PSUM read is
            # a plain copy since stt/ts cannot read PSUM here) ----
            nc.vector.tensor_reduce(
                s1sum, prow[:, 0:NCHUNK], mybir.AxisListType.X, AluOpType.add
            )
            acc_sb = cpool.tile([128, 1], F32, name="acc_sb")
            nc.vector.tensor_copy(acc_sb, psum_acc)
            nc.vector.scalar_tensor_tensor(
                v_sb, s1sum, -float(W), acc_sb,
                AluOpType.mult, AluOpType.add,
            )
            nc.tensor.matmul(psum_tot, v_sb, k2vec, start=True, stop=True)
            nc.vector.tensor_copy(out_sb[0:1, 1:2], psum_tot[0:1, 0:1])

            # ---- small terms: dot/pp/gg off the packed p||g tile ----
            p_ap = pg16[:, 0:64]
            g_ap = pg16[:, 64:128]
            nc.vector.scalar_tensor_tensor(
                junk_stt, p_ap, 1.0, g_ap, AluOpType.mult, AluOpType.mult,
                accum_out=stacked[:, 0:1],
            )
            junk_sq = cpool.tile([128, 64], F32, name="junk_sq")
            nc.scalar.activation(
                junk_sq, p_ap, AF.Square, bias=zeros1,
                accum_out=stacked[:, 1:2],
            )
            nc.scalar.activation(
                junk_sq, g_ap, AF.Square, bias=zeros1,
                accum_out=stacked[:, 2:3],
            )
            nc.tensor.matmul(psum_small, ones32, stacked, start=True, stop=True)

            # ---- scalar chain (partition 0; ts reads PSUM directly;
            # the (1 - .)/8 constant is added on the host) ----
            dot = psum_small[0:1, 0:1]
            pp = psum_small[0:1, 1:2]
            gg = psum_small[0:1, 2:3]
            prodc = sc[0:1, 1:2]
            nc.vector.tensor_scalar(
                prodc, pp, gg, 1e-16, AluOpType.mult, AluOpType.max
            )
            denom = sc[0:1, 7:8]
            nc.scalar.activation(denom, prodc, AF.Sqrt, bias=zeros1[0:1, :])
            inv = sc[0:1, 2:3]
            nc.vector.reciprocal(inv, denom)
            t1 = sc[0:1, 3:4]
            nc.vector.tensor_scalar(
                t1, pp, gg, dot, AluOpType.add, AluOpType.subtract
            )
            t2 = sc[0:1, 4:5]
            nc.vector.tensor_scalar(
                t2, t1, dot, 1.0 / (8.0 * B), AluOpType.subtract, AluOpType.mult
            )
            sims = sc[0:1, 6:7]
            nc.vector.tensor_scalar(
                sims, inv, dot, -1.0 / 8.0, AluOpType.mult, AluOpType.mult
            )
            nc.vector.scalar_tensor_tensor(
                out_sb[0:1, 0:1], sims, 1.0, t2,
                AluOpType.mult, AluOpType.add,
            )
            nc.sync.dma_start(out_d[None, :], out_sb)

    nc.compile()
    return nc


def _pick_w(ps):
    starts = np.arange(0, B, 128)
    for W in W_LADDER:
        s = starts[starts + W < B]
        if s.size == 0 or np.all(ps[s + W] - ps[s + 127] >= MR_BIAS):
            return W
    return B


def kernel(predictions: np.ndarray, correct_output: np.ndarray) -> np.ndarray:
    p = np.ascontiguousarray(np.asarray(predictions, dtype=np.float32))
    g = np.ascontiguousarray(np.asarray(correct_output, dtype=np.float32))

    ps = np.sort(p)
    W = _pick_w(ps)
    if W not in _CACHED:
        _CACHED[W] = _build_nc(W)
    nc = _CACHED[W]

    WIN = 128 * (NCHUNK - 1) + W
    ps16 = np.full(B + WIN, PAD_VAL, dtype=np.float16)
    ps16[:B] = ps.astype(np.float16)
    pg = np.zeros((128, 256), dtype=np.float16)
    pg[:, 0:64] = p.reshape(128, 64)
    pg[:, 64:128] = g.reshape(128, 64)

    in_maps = []
    for k in range(NCORES):
        r0 = k * ROWS_PER_CORE
        in_maps.append(
            {
                "x_win": ps16[r0:r0 + WIN].copy(),
                "p_rows": np.ascontiguousarray(np.concatenate(
                    [ps[r0:r0 + ROWS_PER_CORE].reshape(NCHUNK, 128).T,
                     ps[r0:r0 + ROWS_PER_CORE].reshape(NCHUNK, 128).T
                     + np.float32(MR_BIAS)], axis=1
                )),
                "pg": pg,
            }
        )

    res = None
    last_exc = None
    for _attempt in range(3):
        try:
            res = run_bass_kernel_spmd(nc, in_maps, core_ids=list(range(NCORES)))
            break
        except Exception as e:  # transient NRT/axon device errors
            last_exc = e
            import time as _time
            _time.sleep(1.0)
    if res is None:
        raise last_exc

    # Host gather: per-core scalars + closed-form far-pair constant.
    #   mr = 0.1 - (2/B^2) * (S_near + 0.1*N_far)
    # device out[1] carries -(2/B^2)*S_near_share, out[0] (mse+sim)/8.
    i = np.arange(B, dtype=np.int64)
    hi = np.minimum(128 * (i // 128) + W, B)
    n_near = int(np.sum(hi - i - 1))
    n_far = B * (B - 1) // 2 - n_near
    mr_const = MR_BIAS - 2.0 * MR_BIAS * n_far / (float(B) * float(B))

    total = np.float64(mr_const) + 1.0
    for r in res.results:
        total += np.float64(r["out"][0]) + np.float64(r["out"][1])
    return np.array([total], dtype=np.float32)


if __name__ == "__main__":
    rng = np.random.default_rng(0)
    p = rng.standard_normal(B).astype(np.float32)
    g = rng.standard_normal(B).astype(np.float32)
    print(kernel(p, g))
